# revision 20
# baseline (speedup 1.0000x reference)
"""Self-contained Trainium2 Bass kernel for batched multi-head attention
with interleaved RoPE and a block-causal mask (block size 8).

Shapes (hardcoded): x [8, 1024, 1024] f32, weights [1024, 1024] f32,
freqs_cos/sin [1024, 32] f32 -> out [8, 1024, 1024] f32.

Sharding: data-parallel over batch, one batch element per NeuronCore (8 cores).

v2 design (per core, matmuls in bf16):
  - host pre-transposes x -> XT [D, S]; wq/wk columns permuted so each head's
    64 dims are [32 real | 32 imag]; sin table pre-swapped+signed so that
    rope(t) = t*cos + swap32(t*sinf2).
  - V = XT^T Wv in [S, D] layout with a ones column per head (V' [S, 65]).
  - QT/KT projections per head-pair tile; rope fuses the PSUM->SBUF cast with
    the cos/sin multiplies (tensor_tensor from PSUM), swap32 via SBUF DMA.
  - attention per tile t (heads 2t, 2t+1) is software-pipelined with the QK
    projection of tile t+1 so the PE never idles while ScalarE runs exp:
    scores ST[k, q] per 512-wide piece, both heads' matmuls emitted
    back-to-back at base_partition 0/64 for row-group concurrency; exp (ACT)
    PSUM->SBUF into a persistent staircase buffer; block-diag mask applied
    multiplicatively on GpSimd; PV accumulated per 512-q bank with the two
    q-banks time-sliced through 2 PSUM slots; normalization = DVE
    reciprocal from PSUM + GpSimd partition-broadcast + DVE multiply.
  - final = ot^T Wo, cast on ScalarE (idle in that phase), streamed to HBM.

PSUM budget: 4 banks score pieces + 2 banks PV accumulators + 2 banks
projection chains = 8.
"""

import os
import sys
import types

import numpy as np

B, S, D, H, HD, BS = 8, 1024, 1024, 16, 64, 8
P = 128
NT = D // P  # 8 partition tiles
NCORES = 8
HC = HD + 1  # 65: V columns per head incl. the ones column
PTW = 4608  # staircase width per head: sum_i (1024 - 128 i)
# per-head staircase offsets o[i] = sum_{j<i} (1024 - 128 j)
OFFS = [0, 1024, 1920, 2688, 3328, 3840, 4224, 4480]

LAST_RESULT = None  # BassKernelResults of the most recent run (for test harness)


def _install_axon_hooks():
    """Provide antenv.axon_hooks (NTFF profiling hook) when the image lacks it."""
    if "antenv.axon_hooks" in sys.modules:
        return
    try:
        import antenv
        from trn_agent_boot.trn_boot import _ntff_profile_via_ctypes

        mod = types.ModuleType("antenv.axon_hooks")
        hook = _ntff_profile_via_ctypes("/opt/axon/libaxon_pjrt.so")
        mod.get_axon_ntff_profile_hook = lambda: hook
        mod.set_axon_ntff_profile_hook = lambda h: None
        sys.modules["antenv.axon_hooks"] = mod
        antenv.axon_hooks = mod
    except Exception:
        mod = types.ModuleType("antenv.axon_hooks")
        mod.get_axon_ntff_profile_hook = lambda: None
        mod.set_axon_ntff_profile_hook = lambda h: None
        sys.modules["antenv.axon_hooks"] = mod


_NC_CACHE = {}


def _build_nc():
    """Build and compile the Bass graph (one SPMD program for all 8 cores)."""
    if "nc" in _NC_CACHE:
        return _NC_CACHE["nc"]

    import concourse.mybir as mybir
    import concourse.tile as tile
    from concourse import bacc

    BF = mybir.dt.bfloat16
    F32 = mybir.dt.float32
    MUL = mybir.AluOpType.mult
    ADD = mybir.AluOpType.add
    EXP = mybir.ActivationFunctionType.Exp

    nc = bacc.Bacc("TRN2", target_bir_lowering=False, debug=False)

    xt_d = nc.dram_tensor("xt", [D, S], BF, kind="ExternalInput")
    wq_d = nc.dram_tensor("wq", [D, D], BF, kind="ExternalInput")
    wk_d = nc.dram_tensor("wk", [D, D], BF, kind="ExternalInput")
    wv_d = nc.dram_tensor("wv", [D, D], BF, kind="ExternalInput")
    wo_d = nc.dram_tensor("wo", [D, D], BF, kind="ExternalInput")
    cos_d = nc.dram_tensor("cosf", [P, S], BF, kind="ExternalInput")
    sin_d = nc.dram_tensor("sinf2", [P, S], BF, kind="ExternalInput")
    mask_d = nc.dram_tensor("mask", [P, 2 * P], BF, kind="ExternalInput")
    out_d = nc.dram_tensor("out", [S, D], F32, kind="ExternalOutput")

    with tile.TileContext(nc) as tc:
        with (
            tc.tile_pool(name="big", bufs=1) as big,
            tc.tile_pool(name="rope", bufs=2) as rope_p,
            tc.tile_pool(name="nrm", bufs=2) as nrm,
            tc.tile_pool(name="ob", bufs=3) as ob,
            tc.tile_pool(name="ps_sc", bufs=2, space="PSUM") as ps_sc,
            tc.tile_pool(name="ps_acc", bufs=2, space="PSUM") as ps_acc,
            tc.tile_pool(name="ps_pj", bufs=2, space="PSUM") as ps_pj,
        ):
            xt = [big.tile([P, S], BF, tag=f"xt{j}", name=f"xt{j}") for j in range(NT)]
            wqt = [big.tile([P, D], BF, tag=f"wq{j}", name=f"wq{j}") for j in range(NT)]
            wkt = [big.tile([P, D], BF, tag=f"wk{j}", name=f"wk{j}") for j in range(NT)]
            wvt = [big.tile([P, D], BF, tag=f"wv{j}", name=f"wv{j}") for j in range(NT)]
            wot = [big.tile([P, D], BF, tag=f"wo{j}", name=f"wo{j}") for j in range(NT)]
            qt = [big.tile([P, S], BF, tag=f"qt{t}", name=f"qt{t}") for t in range(NT)]
            kt = [big.tile([P, S], BF, tag=f"kt{t}", name=f"kt{t}") for t in range(NT)]
            vs = [big.tile([P, H * HC], BF, tag=f"vs{t}", name=f"vs{t}") for t in range(NT)]
            ot = [big.tile([P, S], BF, tag=f"ot{t}", name=f"ot{t}") for t in range(NT)]
            cosf = big.tile([P, S], BF, tag="cosf", name="cosf")
            sinf2 = big.tile([P, S], BF, tag="sinf2", name="sinf2")
            maskt = big.tile([P, 2 * P], BF, tag="mask", name="mask")
            pt = big.tile([P, 2 * PTW], BF, tag="pt", name="pt")
            pt3 = pt.rearrange("p (h w) -> p h w", h=2)
            mask3 = maskt.rearrange("p (h w) -> p h w", h=2)

            # ---- input DMA; x first, then wq/wk (gate QK0), wv, wo last ----
            for j in range(NT):
                rs = slice(j * P, (j + 1) * P)
                nc.sync.dma_start(xt[j][:], xt_d[rs, :])
            for j in range(NT):
                rs = slice(j * P, (j + 1) * P)
                nc.sync.dma_start(wqt[j][:], wq_d[rs, :])
            for j in range(NT):
                rs = slice(j * P, (j + 1) * P)
                nc.sync.dma_start(wkt[j][:], wk_d[rs, :])
                nc.sync.dma_start(wvt[j][:], wv_d[rs, :])
            nc.sync.dma_start(cosf[:], cos_d[:])
            nc.sync.dma_start(sinf2[:], sin_d[:])
            nc.sync.dma_start(maskt[:], mask_d[:])
            for j in range(NT):
                rs = slice(j * P, (j + 1) * P)
                nc.sync.dma_start(wot[j][:], wo_d[rs, :])

            # pre-warm the ACT exp table while the PE ramps on projections
            wtile = nrm.tile([1, 32], F32, tag="rec", name="warm")
            nc.vector.memset(wtile[:], 0.0)
            nc.scalar.activation(wtile[:], wtile[:], EXP)

            for t in range(NT):
                nc.vector.memset(
                    vs[t].rearrange("p (h c) -> p h c", c=HC)[:, :, HD : HD + 1], 1.0
                )

            # ---- V projection (natural [S, D] layout, ones col per head);
            # emitted as chunk closures so V out-tiles 2..7 interleave into
            # attention(0)'s step loop (vs[i] lands 2 steps before PV(0, i)
            # needs it)
            def v_chunk(tn, m):
                cs = slice(tn * P, (tn + 1) * P)
                sl = slice(m * 512, (m + 1) * 512)
                pv = ps_pj.tile([P, 512], F32, tag="pj", name="pv")
                for j in range(NT):
                    nc.tensor.matmul(
                        pv[:], xt[j][:, cs], wvt[j][:, sl],
                        start=(j == 0), stop=(j == NT - 1),
                    )
                dst = vs[tn].rearrange("p (h c) -> p h c", c=HC)[
                    :, m * 8 : (m + 1) * 8, 0:HD
                ]
                nc.vector.tensor_copy(dst, pv.rearrange("p (h c) -> p h c", c=HD))

            # ---- QK projection chunks (per target tile, 4 chunks) --------
            # chunk = one m-half of q or k: 8 matmuls + fused cast*cos and
            # cast*sinf2; the second m-half also emits the swap DMAs + add.
            def make_qk_chunks(tn):
                scratch = {}

                def chunk(bufname, m):
                    wt = wqt if bufname == "q" else wkt
                    dst = qt[tn] if bufname == "q" else kt[tn]
                    cs = slice(tn * P, (tn + 1) * P)
                    sl = slice(m * 512, (m + 1) * 512)
                    pp = ps_pj.tile([P, 512], F32, tag="pj", name=f"p{bufname}")
                    for j in range(NT):
                        nc.tensor.matmul(
                            pp[:], wt[j][:, cs], xt[j][:, sl],
                            start=(j == 0), stop=(j == NT - 1),
                        )
                    if m == 0:
                        scratch[bufname] = (
                            rope_p.tile([P, S], BF, tag="bs", name="bs"),
                            rope_p.tile([P, S], BF, tag="bsw", name="bsw"),
                        )
                    bs, bsw = scratch[bufname]
                    nc.vector.tensor_tensor(dst[:, sl], pp[:], cosf[:, sl], op=MUL)
                    nc.vector.tensor_tensor(bs[:, sl], pp[:], sinf2[:, sl], op=MUL)
                    if m == 1:
                        for b4 in range(4):
                            sb = (b4 ^ 1) * 32
                            nc.sync.dma_start(
                                bsw[b4 * 32 : (b4 + 1) * 32, :], bs[sb : sb + 32, :]
                            )
                        nc.vector.tensor_tensor(dst[:], dst[:], bsw[:], op=ADD)

                return [
                    lambda: chunk("q", 0),
                    lambda: chunk("q", 1),
                    lambda: chunk("k", 0),
                    lambda: chunk("k", 1),
                ]

            def pieces(i):
                w = S - 128 * i
                po = 128 * i
                out = [(po, min(512, w))]
                if w > 512:
                    out.append((po + 512, w - 512))
                return out

            def norm(t, hh, jb, acc):
                # NOTE: reciprocal_approx_fast reading PSUM directly returns
                # garbage on HW (sim-only correct) — copy den to SBUF first.
                den = nrm.tile([1, 512], F32, tag="den", name="den")
                nc.scalar.copy(den[:], acc[HD : HD + 1, :])
                rec = nrm.tile([1, 512], F32, tag="rec", name="rec")
                nc.vector.reciprocal_approx_fast(rec[:], den[:])
                bc = nrm.tile([HD, 512], F32, tag="bc", name="bc")
                nc.gpsimd.partition_broadcast(bc[:], rec[:])
                nc.vector.tensor_tensor(
                    ot[t][hh * HD : (hh + 1) * HD, jb * 512 : (jb + 1) * 512],
                    acc[0:HD, :], bc[:], op=MUL,
                )

            def emit_attention(t, chunks):
                chunk_iter = iter(chunks)
                nchunks = len(chunks)
                emitted = [0]

                def drain_chunks(i):
                    # spread chunks evenly across the 8 k-steps
                    target = (i + 1) * nchunks // NT
                    while emitted[0] < target:
                        f = next(chunk_iter, None)
                        if f is None:
                            return
                        f()
                        emitted[0] += 1

                otp = {}
                for hh in (0, 1):
                    otp[(hh, 0)] = ps_acc.tile([HC, 512], F32, tag="acc", name="otp")
                for i in range(NT):
                    oi = OFFS[i]
                    w = S - 128 * i
                    # scores: per-head [128, 1024] slots (bufs=2 ping-pong:
                    # exp(h0, i) overlaps the other head's matmuls); per
                    # piece the two heads' matmuls are adjacent
                    # (base_partition 0/64 -> concurrent row groups); exp is
                    # a single ACT call per (head, k-tile)
                    sps = {}
                    for hh in (0, 1):
                        sps[hh] = ps_sc.tile([P, 1024], F32, tag="sc", name="sc")
                    for (po, pw) in pieces(i):
                        for hh in (0, 1):
                            lo = po - i * P
                            nc.tensor.matmul(
                                sps[hh][:, lo : lo + pw],
                                kt[t][hh * HD : (hh + 1) * HD, i * P : (i + 1) * P],
                                qt[t][hh * HD : (hh + 1) * HD, po : po + pw],
                                start=True, stop=True,
                            )
                    for hh in (0, 1):
                        nc.scalar.activation(
                            pt[:, hh * PTW + oi : hh * PTW + oi + w],
                            sps[hh][:, 0:w], EXP, scale=0.125,
                        )
                    # block-diag mask on the first 128 columns (both heads)
                    nc.vector.tensor_tensor(
                        pt3[:, :, oi : oi + P], pt3[:, :, oi : oi + P],
                        mask3[:, :, :], op=MUL,
                    )
                    # PV q-bank 0 (q in [0, 512)) inline for i <= 3
                    if i <= 3:
                        wd = 512 - 128 * i
                        for hh in (0, 1):
                            h = 2 * t + hh
                            nc.tensor.matmul(
                                otp[(hh, 0)][:, 128 * i : 128 * i + wd],
                                vs[i][:, h * HC : (h + 1) * HC],
                                pt[:, hh * PTW + oi : hh * PTW + oi + wd],
                                start=(i == 0), stop=(i == 3),
                            )
                    if i == 3:
                        for hh in (0, 1):
                            norm(t, hh, 0, otp[(hh, 0)])
                        for hh in (0, 1):
                            otp[(hh, 1)] = ps_acc.tile(
                                [HC, 512], F32, tag="acc", name="otp"
                            )
                        # PV q-bank 1 catch-up over i=0..3 (512 wide each)
                        for ii in range(4):
                            for hh in (0, 1):
                                h = 2 * t + hh
                                lo = hh * PTW + OFFS[ii] + (512 - 128 * ii)
                                nc.tensor.matmul(
                                    otp[(hh, 1)][:, 0:512],
                                    vs[ii][:, h * HC : (h + 1) * HC],
                                    pt[:, lo : lo + 512],
                                    start=(ii == 0), stop=False,
                                )
                    if i >= 4:
                        wd = S - 128 * i
                        for hh in (0, 1):
                            h = 2 * t + hh
                            nc.tensor.matmul(
                                otp[(hh, 1)][:, 128 * i - 512 : 128 * i - 512 + wd],
                                vs[i][:, h * HC : (h + 1) * HC],
                                pt[:, hh * PTW + oi : hh * PTW + oi + wd],
                                start=False, stop=(i == NT - 1),
                            )
                    drain_chunks(i)
                for hh in (0, 1):
                    norm(t, hh, 1, otp[(hh, 1)])

            # ---- main pipeline: QK(0), V(0..1), then attention(t) with
            # V(2..7) interleaved into tile 0 and QK(t+1) into each tile ----
            for f in make_qk_chunks(0):
                f()
            for tn in (0, 1):
                for m in range(2):
                    v_chunk(tn, m)
            for t in range(NT):
                chunks = []
                if t == 0:
                    chunks += [
                        (lambda tn=tn, m=m: v_chunk(tn, m))
                        for tn in range(2, NT)
                        for m in range(2)
                    ]
                if t + 1 < NT:
                    chunks += make_qk_chunks(t + 1)
                emit_attention(t, chunks)

            # ---- output projection: final[s, :] = sum_i ot[i][:, s]^T wo[i]
            for st in range(NT):
                cs = slice(st * P, (st + 1) * P)
                for m in range(2):
                    sl = slice(m * 512, (m + 1) * 512)
                    fp = ps_pj.tile([P, 512], F32, tag="pj", name="fp")
                    for i in range(NT):
                        nc.tensor.matmul(
                            fp[:], ot[i][:, cs], wot[i][:, sl],
                            start=(i == 0), stop=(i == NT - 1),
                        )
                    osb = ob.tile([P, 512], F32, tag="osb", name="osb")
                    nc.scalar.copy(osb[:], fp[:])
                    nc.sync.dma_start(out_d[cs, sl], osb[:])

    nc.compile()
    _NC_CACHE["nc"] = nc
    return nc


def _host_prep(x, wq, wk, wv, wo, freqs_cos, freqs_sin):
    import ml_dtypes

    bf16 = ml_dtypes.bfloat16

    # de-interleave RoPE pairs: permuted col c of head h maps to original
    # column h*64 + (2r if r<32 else 2(r-32)+1)
    r = np.arange(HD)
    src_local = np.where(r < 32, 2 * r, 2 * (r - 32) + 1)
    perm = (np.arange(H)[:, None] * HD + src_local[None, :]).reshape(-1)

    wq_p = np.ascontiguousarray(wq[:, perm]).astype(bf16)
    wk_p = np.ascontiguousarray(wk[:, perm]).astype(bf16)
    wv_c = np.ascontiguousarray(wv).astype(bf16)
    wo_c = np.ascontiguousarray(wo).astype(bf16)

    cos_t = np.ascontiguousarray(freqs_cos.T).astype(np.float32)  # [32, S]
    sin_t = np.ascontiguousarray(freqs_sin.T).astype(np.float32)
    cosf = np.concatenate([cos_t, cos_t, cos_t, cos_t], 0).astype(bf16)  # [128,S]
    # sinf2 = swap32(signed sin table [-s, s, -s, s]) = [s, -s, s, -s] so that
    # rope(t) = t*cos + swap32(t*sinf2)
    sinf2 = np.concatenate([sin_t, -sin_t, sin_t, -sin_t], 0).astype(bf16)

    kq = np.arange(P)
    mask = ((kq[:, None] // BS) <= (kq[None, :] // BS)).astype(bf16)  # [128,128]
    mask2 = np.concatenate([mask, mask], 1)  # [128, 256]: one copy per head

    in_maps = []
    for b in range(NCORES):
        xt = np.ascontiguousarray(x[b].T).astype(bf16)  # [D, S]
        in_maps.append(
            {
                "xt": xt,
                "wq": wq_p,
                "wk": wk_p,
                "wv": wv_c,
                "wo": wo_c,
                "cosf": cosf,
                "sinf2": sinf2,
                "mask": mask2,
            }
        )
    return in_maps


def kernel(x, wq, wk, wv, wo, freqs_cos, freqs_sin):
    global LAST_RESULT
    x = np.asarray(x, dtype=np.float32)
    wq = np.asarray(wq, dtype=np.float32)
    wk = np.asarray(wk, dtype=np.float32)
    wv = np.asarray(wv, dtype=np.float32)
    wo = np.asarray(wo, dtype=np.float32)
    freqs_cos = np.asarray(freqs_cos, dtype=np.float32)
    freqs_sin = np.asarray(freqs_sin, dtype=np.float32)

    trace = bool(os.environ.get("BASS_TRACE"))
    if trace:
        _install_axon_hooks()
        import concourse.bass_utils as bass_utils

        bass_utils.upload_artifacts = lambda tmpdir: tmpdir  # no-egress sandbox

    from concourse.bass_utils import run_bass_kernel_spmd

    nc = _build_nc()
    in_maps = _host_prep(x, wq, wk, wv, wo, freqs_cos, freqs_sin)
    res = run_bass_kernel_spmd(
        nc, in_maps, core_ids=list(range(NCORES)), trace=trace
    )
    LAST_RESULT = res
    out = np.stack([res.results[b]["out"] for b in range(NCORES)], 0)
    return out.astype(np.float32)
